# revision 17
# baseline (speedup 1.0000x reference)
"""Trainium2 Bass kernel for nn_CAM_62852551409742 (low-rank tanh rewrite).

Math (reference):
  f = feats[:, :, 0, :]                               [R,B,T], R=4, B=512, T=150
  v = feats.reshape(B, K)                             [B,K], K=600
  att[r,b,t,k] = tanh(u[r,b,t] * v[b,k]),  u = a[r]*f
  Hm = relu(att @ Wc[r].T + f*W[r])                   [R,B,T,32]
  attf = Hm @ Wh[r] + f
  out = (attf-cat @ W1.T + b1) @ W2.T + b2            [B,1,7]

Key rewrite: |u*v| <= ~1.4 on this data, so tanh(uv) ~= sum_j c_j (uv)^p_j
(odd powers p=1,3,5,7,9; weighted-LS fit, err ~1e-4 diluted to ~1e-5 at out).
Then  att @ Wc.T = sum_j u^p_j * S_j,
      S_j[r,b,c] = sum_k c_j v[b,k]^p_j Wc[r,c,k]
which kills the 184M-element tanh and the K=600 contraction entirely. The
f*W[r] term folds in as an extra contraction row: S_0 += W[r,:]/a[r]
(phi_0 = u = a*f), added via an indicator row in the kt=4 tile.

Per core (64 batches), bf16 operands with fp32 PSUM accumulation:
  stage D: S_nat[(rc),(bt,jf)] via 5 MMs (fixed wct stationary, vp moving),
           4 PE transposes -> [8*btloc+jf x (rc)], then flat-address DMA
           realign into s_all rows 32q+jf (LDW needs 32-aligned bases)
  stage A: per (b,r) col-tiled MMs  Hm[(rc),t] = sum_jf S_jf * phi_jf
  relu:    DVE/ACT alternating, PSUM -> SBUF bf16
  final:   per-t MMs vs U[(rc),t,i]=Wh*Wx in 4 col groups + fp32 f@Wx tail
"""

import os
from contextlib import ExitStack

import numpy as np
import ml_dtypes

import concourse.bacc as bacc
import concourse.bass as bass
import concourse.tile as tile
from concourse import mybir
from concourse import bass_utils

R, B, T, H = 4, 512, 150, 32
K = R * T                      # 600
NCORES = 8
BL = B // NCORES               # 64 batches per core
J = 5                          # odd powers 1..9
JF = 5
POWS = (1, 3, 5, 7, 9)
KTS = [(0, 128), (128, 128), (256, 128), (384, 128), (512, 88)]
KPD = [128, 128, 128, 128, 89]   # stage-D contraction (kt4 + indicator row)
F32 = mybir.dt.float32
BF16 = mybir.dt.bfloat16
BF = ml_dtypes.bfloat16

_CACHE = {}


def build_nc():
    nc = bacc.Bacc("TRN2", target_bir_lowering=False)
    phi_d = nc.dram_tensor("phi", [JF, R * BL * T], BF16, kind="ExternalInput")
    vpn_d = nc.dram_tensor("vpn", [128, 5, 512], BF16, kind="ExternalInput")
    wct_d = nc.dram_tensor("wct", [128, 5, 128], BF16, kind="ExternalInput")
    id_d = nc.dram_tensor("idm", [128, 128], BF16, kind="ExternalInput")
    u_d = nc.dram_tensor("u", [128, T, 7], BF16, kind="ExternalInput")
    ft_d = nc.dram_tensor("ft", [128, 5, BL], F32, kind="ExternalInput")
    wx_d = nc.dram_tensor("wx", [128, 5, 7], F32, kind="ExternalInput")
    bx_d = nc.dram_tensor("bx", [7, 1], F32, kind="ExternalInput")
    out_d = nc.dram_tensor("out", [7, BL], F32, kind="ExternalOutput")
    dbg = os.environ.get("KDEBUG")
    if dbg:
        dsn_d = nc.dram_tensor("dsn", [128, 512], BF16, kind="ExternalOutput")
        dst_d = nc.dram_tensor("dst", [128, 512], BF16, kind="ExternalOutput")
        dsa_d = nc.dram_tensor("dsa", [128, 2048], BF16, kind="ExternalOutput")

    with tile.TileContext(nc) as tc, ExitStack() as ctx:
        consts = ctx.enter_context(tc.tile_pool(name="consts", bufs=1))
        hmp = ctx.enter_context(tc.tile_pool(name="hm", bufs=1))
        ps_s = ctx.enter_context(tc.tile_pool(name="pss", bufs=1, space="PSUM"))
        ps_t = ctx.enter_context(tc.tile_pool(name="pst", bufs=2, space="PSUM"))
        ps_hm = ctx.enter_context(tc.tile_pool(name="psh", bufs=4, space="PSUM"))
        ps_o = ctx.enter_context(tc.tile_pool(name="pso", bufs=1, space="PSUM"))

        vpn_sb = consts.tile([128, 5, 512], BF16)
        wct_sb = consts.tile([128, 5, 128], BF16)
        id_sb = consts.tile([128, 128], BF16)
        phi_sb = consts.tile([128, R * BL * T], BF16)
        snat_sb = consts.tile([128, 512], BF16)
        st_sb = consts.tile([128, 512], BF16)
        s_all = consts.tile([128, 2048], BF16)
        u_sb = consts.tile([128, T, 7], BF16)
        ft_sb = consts.tile([128, 5, BL], F32)
        wx_sb = consts.tile([128, 5, 7], F32)
        bx_sb = consts.tile([7, 1], F32)
        hm_sb = hmp.tile([128, BL * T], BF16)
        tiny = consts.tile([1, 1], F32)

        # stage-D inputs first; phi rows split into per-r column chunks so
        # descriptors spread across queues (per-descriptor DMA bw is low)
        nc.scalar.dma_start(out=wct_sb[:], in_=wct_d[:])
        nc.scalar.dma_start(out=vpn_sb[:], in_=vpn_d[:])
        nc.scalar.dma_start(out=id_sb[:], in_=id_d[:])
        CH = R * BL * T // 4
        for cc in range(4):
            nc.sync.dma_start(out=phi_sb[0:JF, cc * CH:(cc + 1) * CH],
                              in_=phi_d[:, cc * CH:(cc + 1) * CH])
            nc.gpsimd.dma_start(out=phi_sb[32:32 + JF, cc * CH:(cc + 1) * CH],
                                in_=phi_d[:, cc * CH:(cc + 1) * CH])
        for cc in range(4):
            nc.sync.dma_start(out=phi_sb[64:64 + JF, cc * CH:(cc + 1) * CH],
                              in_=phi_d[:, cc * CH:(cc + 1) * CH])
            nc.gpsimd.dma_start(out=phi_sb[96:96 + JF, cc * CH:(cc + 1) * CH],
                                in_=phi_d[:, cc * CH:(cc + 1) * CH])
        nc.scalar.dma_start(out=u_sb[:], in_=u_d[:])
        nc.scalar.dma_start(out=ft_sb[:], in_=ft_d[:])
        nc.scalar.dma_start(out=wx_sb[:], in_=wx_d[:])
        nc.scalar.dma_start(out=bx_sb[:], in_=bx_d[:])
        # preload ACT's table set (has Relu) while DMAs run
        nc.vector.memset(tiny[:], 0.0)
        nc.scalar.activation(out=tiny[:], in_=tiny[:],
                             func=mybir.ActivationFunctionType.Relu)

        hm3 = hm_sb.rearrange("p (b t) -> p b t", t=T)

        # ---- stage D: S_nat = wct.T @ vpn, all 64 batches in one bank ----
        sp = ps_s.tile([128, 512], F32)
        for kt in range(5):
            kp = KPD[kt]
            nc.tensor.matmul(out=sp[:], lhsT=wct_sb[0:kp, kt, :],
                             rhs=vpn_sb[0:kp, kt, :],
                             start=(kt == 0), stop=(kt == 4))
        nc.vector.tensor_copy(snat_sb[:], sp[:])

        # transpose chunks then DMA-realign rows (8*btloc+jf) -> (32q+jf)
        engs = [nc.sync, nc.gpsimd, nc.scalar]
        for C in range(4):
            tp = ps_t.tile([128, 128], BF16, tag="tp", padded_shape=[None, 1024])
            nc.tensor.transpose(out=tp[:], in_=snat_sb[:, 128 * C:128 * (C + 1)],
                                identity=id_sb[:])
            nc.vector.tensor_copy(st_sb[:, 128 * C:128 * (C + 1)], tp[:])
            for q in range(4):
                for mt in range(4):
                    m = 4 * C + mt
                    engs[(4 * C + q) % 3].dma_start(
                        out=s_all[32 * q:32 * q + JF,
                                  128 * m:128 * m + 128],
                        in_=st_sb[32 * q + 8 * mt:32 * q + 8 * mt + JF,
                                  128 * C:128 * C + 128],
                    )

        if dbg:
            nc.sync.dma_start(out=dsn_d[:], in_=snat_sb[:])
            nc.sync.dma_start(out=dst_d[:], in_=st_sb[:])
            nc.sync.dma_start(out=dsa_d[:], in_=s_all[:])

        state = {"flip": False}

        def stage_a(bs):
            """Hm for a chunk of up to 3 batches (same q, consecutive m)."""
            pt = ps_hm.tile([128, 512], F32, tag="hmps")
            for slot, b in enumerate(bs):
                q, m = b % 4, b // 4
                for r in range(R):
                    nc.tensor.matmul(
                        out=pt[32 * r:32 * (r + 1),
                               150 * slot:150 * slot + 150],
                        lhsT=s_all[32 * q:32 * q + JF,
                                   128 * m + 32 * r:128 * m + 32 * r + 32],
                        rhs=phi_sb[32 * q:32 * q + JF,
                                   (r * BL + b) * T:(r * BL + b) * T + T],
                        start=True, stop=True,
                        tile_position=(32 * q, 32 * r),
                        skip_group_check=True,
                    )
            # relu chunk: PSUM fp32 -> SBUF bf16, strided over b (step 4)
            o = hm3[:, bs[0]:bs[-1] + 1:4, :]
            state["flip"] = not state["flip"]
            if state["flip"]:
                nc.vector.tensor_scalar_max(
                    out=o, in0=pt[:, 0:150 * len(bs)], scalar1=0.0)
            else:
                nc.scalar.activation(
                    out=o, in_=pt[:, 0:150 * len(bs)],
                    func=mybir.ActivationFunctionType.Relu)

        # q-major batch order (chunks of 3 within a pass share a psum bank)
        for q in range(4):
            for i in range(0, 16, 3):
                ms = list(range(16))[i:i + 3]
                stage_a([4 * m + q for m in ms])

        # final: out[i,b] = sum_{(rc),t} relu(Hm)*U + sum f*Wx + bx
        op = ps_o.tile([128, BL], F32, padded_shape=[None, 512])
        glast = {gg: max(t for t in range(T) if t % 4 == gg) for gg in range(4)}
        for t in range(T):
            g = t % 4
            nc.tensor.matmul(
                out=op[32 * g:32 * g + 7, :],
                lhsT=u_sb[:, t, :],
                rhs=hm3[:, :, t],
                start=(t < 4),
                stop=(g != 0 and t == glast[g]),
                tile_position=(0, 32 * g),
                skip_group_check=True,
            )
        for kt, (k0, kp) in enumerate(KTS):
            nc.tensor.matmul(
                out=op[0:7, :],
                lhsT=wx_sb[0:kp, kt, :],
                rhs=ft_sb[0:kp, kt, :],
                start=False, stop=(kt == 4),
                tile_position=(0, 0),
                skip_group_check=True,
            )

        # tail: sum the 4 col groups + bias
        c1 = consts.tile([7, BL], F32)
        c2 = consts.tile([7, BL], F32)
        s1 = consts.tile([7, BL], F32)
        s2 = consts.tile([7, BL], F32)
        ob = consts.tile([7, BL], F32)
        nc.vector.tensor_copy(c1[:], op[32:39, :])
        nc.scalar.copy(c2[:], op[96:103, :])
        nc.vector.scalar_tensor_tensor(
            out=s1[:], in0=op[0:7, :], scalar=bx_sb[:], in1=c1[:],
            op0=mybir.AluOpType.add, op1=mybir.AluOpType.add)
        nc.vector.scalar_tensor_tensor(
            out=s2[:], in0=op[64:71, :], scalar=0.0, in1=c2[:],
            op0=mybir.AluOpType.add, op1=mybir.AluOpType.add)
        nc.vector.tensor_add(ob[:], s1[:], s2[:])
        nc.sync.dma_start(out=out_d[:], in_=ob[:])

    nc.finalize()
    return nc


def _fit_poly(u, v):
    xmax = float(np.abs(u).max()) * float(np.abs(v).max()) * 1.02 + 1e-30
    xs = xmax * np.sin(np.linspace(-np.pi / 2, np.pi / 2, 4001))
    A = xs[:, None] ** np.array(POWS)[None, :]
    w = 1.0 / (0.05 + np.abs(xs))
    coef, *_ = np.linalg.lstsq(A * w[:, None], np.tanh(xs) * w, rcond=None)
    return coef


def _host_prep(feats, a, W, Wc, Wh, W1, b1, W2, b2):
    f = feats[:, :, 0, :]                              # [R,B,T]
    u = a[:, None, None] * f                           # [R,B,T]
    v = feats.reshape(B, K)                            # [B,K]
    coef = _fit_poly(u, v)
    Wx = W2 @ W1                                       # [7,K]
    bx = W2 @ b1 + b2                                  # [7]

    # U[(rc), t, i] = Wh[r,c] * Wx[i, r*T+t]
    U = np.zeros((128, T, 7), np.float32)
    for r in range(R):
        blk = Wx[:, r * T:(r + 1) * T].T               # [T,7]
        U[r * H:(r + 1) * H] = Wh[r][:, None, None] * blk[None]

    # wct[k, kt, 32r+c] = Wc[r, c, k0+k]; row 88 of kt4 = W/a (f*W fold)
    wct = np.zeros((128, 5, 128), np.float32)
    for kt, (k0, kp) in enumerate(KTS):
        for r in range(R):
            wct[:kp, kt, 32 * r:32 * (r + 1)] = Wc[r, :, k0:k0 + kp].T
    wct[88, 4, :] = (W / a[:, None]).reshape(128)

    wx_h = np.zeros((128, 5, 7), np.float32)
    for kt, (k0, kp) in enumerate(KTS):
        wx_h[:kp, kt, :] = Wx[:, k0:k0 + kp].T

    fT_full = np.concatenate([f[r].T for r in range(R)], axis=0)  # [K, B]

    # basis powers with the tanh-poly coefficients folded into the v side
    vbasis = np.stack([coef[j] * v ** POWS[j] for j in range(J)], 0)  # [J,B,K]
    ubasis = np.stack([u ** POWS[j] for j in range(J)], 0)            # [J,R,B,T]

    # bt -> b permutation: bt = 16C+4q+Mt carries batch b = 16C+4Mt+q, so
    # the PE transpose drops rows at partition 32q+8Mt+jf for chunk-local Mt
    bt2b = np.zeros(BL, np.int64)
    for bt in range(BL):
        Cc, rm = bt // 16, bt % 16
        bt2b[bt] = 16 * Cc + 4 * (rm % 4) + rm // 4

    idm = np.eye(128, dtype=np.float32)

    in_maps = []
    for mcore in range(NCORES):
        b0 = mcore * BL
        phi = ubasis[:, :, b0:b0 + BL, :].reshape(J, R * BL * T)
        # vpn[k, kt, 8*bt+jf] = vbasis[jf, b0+bt2b[bt], k0+k]; kt4 row 88
        # is the indicator for the f*W fold (pairs with wct row 88)
        vpn = np.zeros((128, 5, 512), np.float32)
        for kt, (k0, kp) in enumerate(KTS):
            vb = vbasis[:, b0 + bt2b, k0:k0 + kp]      # [J, BL, kp]
            arr = np.zeros((kp, BL, 8), np.float32)
            arr[:, :, :J] = vb.transpose(2, 1, 0)
            vpn[:kp, kt, :] = arr.reshape(kp, 512)
        vpn[88, 4, 0::8] = 1.0
        ft_h = np.zeros((128, 5, BL), np.float32)
        for kt, (k0, kp) in enumerate(KTS):
            ft_h[:kp, kt, :] = fT_full[k0:k0 + kp, b0:b0 + BL]
        in_maps.append({
            "phi": phi.astype(BF),
            "vpn": vpn.astype(BF),
            "wct": wct.astype(BF),
            "idm": idm.astype(BF),
            "u": U.astype(BF),
            "ft": ft_h,
            "wx": wx_h,
            "bx": bx.astype(np.float32).reshape(7, 1),
        })
    return in_maps


def kernel(feats_list, a, W, Wc, Wh, W1, b1, W2, b2):
    feats = np.asarray(feats_list, np.float32)
    in_maps = _host_prep(
        feats,
        np.asarray(a, np.float32),
        np.asarray(W, np.float32),
        np.asarray(Wc, np.float32),
        np.asarray(Wh, np.float32),
        np.asarray(W1, np.float32),
        np.asarray(b1, np.float32),
        np.asarray(W2, np.float32),
        np.asarray(b2, np.float32),
    )
    if "nc" not in _CACHE:
        _CACHE["nc"] = build_nc()
    res = bass_utils.run_bass_kernel_spmd(
        _CACHE["nc"], in_maps, core_ids=list(range(NCORES))
    )
    _CACHE["last_result"] = res
    out = np.concatenate([r["out"].T for r in res.results], axis=0)  # [B,7]
    return out[:, None, :].astype(np.float32)                        # [B,1,7]


# revision 18
# speedup vs baseline: 1.4368x; 1.4368x over previous
"""Trainium2 Bass kernel for nn_CAM_62852551409742 (low-rank tanh rewrite).

Math (reference):
  f = feats[:, :, 0, :]                               [R,B,T], R=4, B=512, T=150
  v = feats.reshape(B, K)                             [B,K], K=600
  att[r,b,t,k] = tanh(u[r,b,t] * v[b,k]),  u = a[r]*f
  Hm = relu(att @ Wc[r].T + f*W[r])                   [R,B,T,32]
  attf = Hm @ Wh[r] + f
  out = (attf-cat @ W1.T + b1) @ W2.T + b2            [B,1,7]

Key rewrite: |u*v| <= ~1.4 on this data, so tanh(uv) ~= sum_j c_j (uv)^p_j
(odd powers p=1,3,5,7,9; weighted-LS fit). Then
  att @ Wc.T = sum_j u^p_j * S_j,
  S_j[r,b,c] = sum_k c_j v[b,k]^p_j Wc[r,c,k]
which kills the 184M-element tanh and the K=600 contraction entirely. The
f*W[r] term folds in as an extra contraction row: S_0 += W[r,:]/a[r]
(since phi_0 = u = a*f), via an indicator row in the kt=4 tile.

Per core (64 batches), bf16 operands with fp32 PSUM accumulation:
  stage D: S_nat[(rc),(b,jf)] via 5 MMs (fixed wct stationary, vpn moving),
           4 PE transposes -> [(8*bloc+jf) x (rc)] chunks, then 64 small
           DMA realigns into s_all[jf, 128b + (rc)] (partitions 0..4)
  stage A: per (b,r) col-tiled MMs  Hm[(rc),t] = sum_jf S_jf * phi_jf,
           all operands at base partition 0, tile_position (0, 32r)
  relu:    DVE/ACT alternating, PSUM -> SBUF bf16
  final:   per-t MMs vs U[(rc),t,i]=Wh*Wx in 4 col groups + fp32 f@Wx tail
"""

import os
from contextlib import ExitStack

import numpy as np
import ml_dtypes

import concourse.bacc as bacc
import concourse.bass as bass
import concourse.tile as tile
from concourse import mybir
from concourse import bass_utils

R, B, T, H = 4, 512, 150, 32
K = R * T                      # 600
NCORES = 8
BL = B // NCORES               # 64 batches per core
J = 5                          # odd powers 1..9
JF = 5
POWS = (1, 3, 5, 7, 9)
KTS = [(0, 128), (128, 128), (256, 128), (384, 128), (512, 88)]
KPD = [128, 128, 128, 128, 89]   # stage-D contraction (kt4 + indicator row)
F32 = mybir.dt.float32
BF16 = mybir.dt.bfloat16
BF = ml_dtypes.bfloat16

_CACHE = {}


def build_nc():
    nc = bacc.Bacc("TRN2", target_bir_lowering=False)
    phi_d = nc.dram_tensor("phi", [JF, R * BL * T], BF16, kind="ExternalInput")
    vpn_d = nc.dram_tensor("vpn", [128, 5, 512], BF16, kind="ExternalInput")
    wct_d = nc.dram_tensor("wct", [128, 5, 128], BF16, kind="ExternalInput")
    id_d = nc.dram_tensor("idm", [128, 128], BF16, kind="ExternalInput")
    u_d = nc.dram_tensor("u", [128, T, 7], BF16, kind="ExternalInput")
    ft_d = nc.dram_tensor("ft", [128, 5, BL], F32, kind="ExternalInput")
    wx_d = nc.dram_tensor("wx", [128, 5, 7], F32, kind="ExternalInput")
    bx_d = nc.dram_tensor("bx", [7, 1], F32, kind="ExternalInput")
    out_d = nc.dram_tensor("out", [7, BL], F32, kind="ExternalOutput")
    dbg = os.environ.get("KDEBUG")
    if dbg:
        dsn_d = nc.dram_tensor("dsn", [128, 512], BF16, kind="ExternalOutput")
        dst_d = nc.dram_tensor("dst", [128, 512], BF16, kind="ExternalOutput")
        dsa_d = nc.dram_tensor("dsa", [8, 8192], BF16, kind="ExternalOutput")

    with tile.TileContext(nc) as tc, ExitStack() as ctx:
        consts = ctx.enter_context(tc.tile_pool(name="consts", bufs=1))
        hmp = ctx.enter_context(tc.tile_pool(name="hm", bufs=1))
        ps_s = ctx.enter_context(tc.tile_pool(name="pss", bufs=1, space="PSUM"))
        ps_t = ctx.enter_context(tc.tile_pool(name="pst", bufs=2, space="PSUM"))
        ps_hm = ctx.enter_context(tc.tile_pool(name="psh", bufs=4, space="PSUM"))
        ps_o = ctx.enter_context(tc.tile_pool(name="pso", bufs=1, space="PSUM"))

        vpn_sb = consts.tile([128, 5, 512], BF16)
        wct_sb = consts.tile([128, 5, 128], BF16)
        id_sb = consts.tile([128, 128], BF16)
        phi_sb = consts.tile([8, R * BL * T], BF16)
        snat_sb = consts.tile([128, 512], BF16)
        st_sb = consts.tile([128, 512], BF16)
        s_all = consts.tile([8, 128 * BL], BF16)
        u_sb = consts.tile([128, T, 7], BF16)
        ft_sb = consts.tile([128, 5, BL], F32)
        wx_sb = consts.tile([128, 5, 7], F32)
        bx_sb = consts.tile([7, 1], F32)
        hm_sb = hmp.tile([128, BL * T], BF16)
        tiny = consts.tile([1, 1], F32)

        # stage-D inputs first; phi in (r, b-half) pieces, rotated across
        # the three DMA-capable engine rings for queue parallelism
        engs = [nc.sync, nc.gpsimd, nc.scalar]
        nc.scalar.dma_start(out=wct_sb[:], in_=wct_d[:])
        nc.scalar.dma_start(out=vpn_sb[:], in_=vpn_d[:])
        nc.scalar.dma_start(out=id_sb[:], in_=id_d[:])
        HBT = BL * T // 2
        ei = 0
        for half in range(2):
            for r in range(R):
                lo = r * BL * T + half * HBT
                engs[ei % 2].dma_start(out=phi_sb[0:JF, lo:lo + HBT],
                                       in_=phi_d[:, lo:lo + HBT])
                ei += 1
        nc.gpsimd.dma_start(out=u_sb[:], in_=u_d[:])
        nc.sync.dma_start(out=ft_sb[:], in_=ft_d[:])
        nc.gpsimd.dma_start(out=wx_sb[:], in_=wx_d[:])
        nc.sync.dma_start(out=bx_sb[:], in_=bx_d[:])
        # preload ACT's table set (has Relu) while DMAs run
        nc.vector.memset(tiny[:], 0.0)
        nc.scalar.activation(out=tiny[:], in_=tiny[:],
                             func=mybir.ActivationFunctionType.Relu)

        hm3 = hm_sb.rearrange("p (b t) -> p b t", t=T)

        # ---- stage D: S_nat = wct.T @ vpn, all 64 batches in one bank ----
        sp = ps_s.tile([128, 512], F32)
        for kt in range(5):
            kp = KPD[kt]
            nc.tensor.matmul(out=sp[:], lhsT=wct_sb[0:kp, kt, :],
                             rhs=vpn_sb[0:kp, kt, :],
                             start=(kt == 0), stop=(kt == 4))
        nc.vector.tensor_copy(snat_sb[:], sp[:])

        # transpose chunks then DMA-realign rows (8*bloc+jf) -> (jf)
        for C in range(4):
            tp = ps_t.tile([128, 128], BF16, tag="tp", padded_shape=[None, 1024])
            nc.tensor.transpose(out=tp[:], in_=snat_sb[:, 128 * C:128 * (C + 1)],
                                identity=id_sb[:])
            nc.vector.tensor_copy(st_sb[:, 128 * C:128 * (C + 1)], tp[:])
            for bloc in range(16):
                b = 16 * C + bloc
                engs[(C + bloc) % 3].dma_start(
                    out=s_all[0:JF, 128 * b:128 * b + 128],
                    in_=st_sb[8 * bloc:8 * bloc + JF, 128 * C:128 * C + 128],
                )

        if dbg:
            nc.sync.dma_start(out=dsn_d[:], in_=snat_sb[:])
            nc.sync.dma_start(out=dst_d[:], in_=st_sb[:])
            nc.sync.dma_start(out=dsa_d[:], in_=s_all[:])

        state = {"flip": False}

        def stage_a(bs):
            """Hm for a chunk of up to 3 consecutive batches."""
            pt = ps_hm.tile([128, 512], F32, tag="hmps")
            for slot, b in enumerate(bs):
                for r in range(R):
                    nc.tensor.matmul(
                        out=pt[32 * r:32 * (r + 1),
                               150 * slot:150 * slot + 150],
                        lhsT=s_all[0:JF,
                                   128 * b + 32 * r:128 * b + 32 * r + 32],
                        rhs=phi_sb[0:JF,
                                   (r * BL + b) * T:(r * BL + b) * T + T],
                        start=True, stop=True,
                        tile_position=(0, 32 * r),
                        skip_group_check=True,
                    )
            # relu chunk: PSUM fp32 -> SBUF bf16 (consecutive b's)
            o = hm3[:, bs[0]:bs[-1] + 1, :]
            state["flip"] = not state["flip"]
            if state["flip"]:
                nc.vector.tensor_scalar_max(
                    out=o, in0=pt[:, 0:150 * len(bs)], scalar1=0.0)
            else:
                nc.scalar.activation(
                    out=o, in_=pt[:, 0:150 * len(bs)],
                    func=mybir.ActivationFunctionType.Relu)

        for i in range(0, BL, 3):
            stage_a(list(range(i, min(i + 3, BL))))

        # final: out[i,b] = sum_{(rc),t} relu(Hm)*U + sum f*Wx + bx
        op = ps_o.tile([128, BL], F32, padded_shape=[None, 512])
        glast = {gg: max(t for t in range(T) if t % 4 == gg) for gg in range(4)}
        for t in range(T):
            g = t % 4
            nc.tensor.matmul(
                out=op[32 * g:32 * g + 7, :],
                lhsT=u_sb[:, t, :],
                rhs=hm3[:, :, t],
                start=(t < 4),
                stop=(g != 0 and t == glast[g]),
                tile_position=(0, 32 * g),
                skip_group_check=True,
            )
        for kt, (k0, kp) in enumerate(KTS):
            nc.tensor.matmul(
                out=op[0:7, :],
                lhsT=wx_sb[0:kp, kt, :],
                rhs=ft_sb[0:kp, kt, :],
                start=False, stop=(kt == 4),
                tile_position=(0, 0),
                skip_group_check=True,
            )

        # tail: sum the 4 col groups + bias
        c1 = consts.tile([7, BL], F32)
        c2 = consts.tile([7, BL], F32)
        s1 = consts.tile([7, BL], F32)
        s2 = consts.tile([7, BL], F32)
        ob = consts.tile([7, BL], F32)
        nc.vector.tensor_copy(c1[:], op[32:39, :])
        nc.scalar.copy(c2[:], op[96:103, :])
        nc.vector.scalar_tensor_tensor(
            out=s1[:], in0=op[0:7, :], scalar=bx_sb[:], in1=c1[:],
            op0=mybir.AluOpType.add, op1=mybir.AluOpType.add)
        nc.vector.scalar_tensor_tensor(
            out=s2[:], in0=op[64:71, :], scalar=0.0, in1=c2[:],
            op0=mybir.AluOpType.add, op1=mybir.AluOpType.add)
        nc.vector.tensor_add(ob[:], s1[:], s2[:])
        nc.sync.dma_start(out=out_d[:], in_=ob[:])

    nc.finalize()
    return nc


def _fit_poly(u, v):
    xmax = float(np.abs(u).max()) * float(np.abs(v).max()) * 1.02 + 1e-30
    xs = xmax * np.sin(np.linspace(-np.pi / 2, np.pi / 2, 4001))
    A = xs[:, None] ** np.array(POWS)[None, :]
    w = 1.0 / (0.05 + np.abs(xs))
    coef, *_ = np.linalg.lstsq(A * w[:, None], np.tanh(xs) * w, rcond=None)
    return coef


def _host_prep(feats, a, W, Wc, Wh, W1, b1, W2, b2):
    f = feats[:, :, 0, :]                              # [R,B,T]
    u = a[:, None, None] * f                           # [R,B,T]
    v = feats.reshape(B, K)                            # [B,K]
    coef = _fit_poly(u, v)
    Wx = W2 @ W1                                       # [7,K]
    bx = W2 @ b1 + b2                                  # [7]

    # U[(rc), t, i] = Wh[r,c] * Wx[i, r*T+t]
    U = np.zeros((128, T, 7), np.float32)
    for r in range(R):
        blk = Wx[:, r * T:(r + 1) * T].T               # [T,7]
        U[r * H:(r + 1) * H] = Wh[r][:, None, None] * blk[None]

    # wct[k, kt, 32r+c] = Wc[r, c, k0+k]; row 88 of kt4 = W/a (f*W fold)
    wct = np.zeros((128, 5, 128), np.float32)
    for kt, (k0, kp) in enumerate(KTS):
        for r in range(R):
            wct[:kp, kt, 32 * r:32 * (r + 1)] = Wc[r, :, k0:k0 + kp].T
    wct[88, 4, :] = (W / a[:, None]).reshape(128)

    wx_h = np.zeros((128, 5, 7), np.float32)
    for kt, (k0, kp) in enumerate(KTS):
        wx_h[:kp, kt, :] = Wx[:, k0:k0 + kp].T

    fT_full = np.concatenate([f[r].T for r in range(R)], axis=0)  # [K, B]

    # basis powers with the tanh-poly coefficients folded into the v side
    vbasis = np.stack([coef[j] * v ** POWS[j] for j in range(J)], 0)  # [J,B,K]
    ubasis = np.stack([u ** POWS[j] for j in range(J)], 0)            # [J,R,B,T]

    idm = np.eye(128, dtype=np.float32)

    in_maps = []
    for mcore in range(NCORES):
        b0 = mcore * BL
        phi = ubasis[:, :, b0:b0 + BL, :].reshape(J, R * BL * T)
        # vpn[k, kt, 8*b+jf] = vbasis[jf, b0+b, k0+k]; kt4 row 88 is the
        # indicator for the f*W fold (pairs with wct row 88 = W/a)
        vpn = np.zeros((128, 5, 512), np.float32)
        for kt, (k0, kp) in enumerate(KTS):
            vb = vbasis[:, b0:b0 + BL, k0:k0 + kp]     # [J, BL, kp]
            arr = np.zeros((kp, BL, 8), np.float32)
            arr[:, :, :J] = vb.transpose(2, 1, 0)
            vpn[:kp, kt, :] = arr.reshape(kp, 512)
        vpn[88, 4, 0::8] = 1.0
        ft_h = np.zeros((128, 5, BL), np.float32)
        for kt, (k0, kp) in enumerate(KTS):
            ft_h[:kp, kt, :] = fT_full[k0:k0 + kp, b0:b0 + BL]
        in_maps.append({
            "phi": phi.astype(BF),
            "vpn": vpn.astype(BF),
            "wct": wct.astype(BF),
            "idm": idm.astype(BF),
            "u": U.astype(BF),
            "ft": ft_h,
            "wx": wx_h,
            "bx": bx.astype(np.float32).reshape(7, 1),
        })
    return in_maps


def kernel(feats_list, a, W, Wc, Wh, W1, b1, W2, b2):
    feats = np.asarray(feats_list, np.float32)
    in_maps = _host_prep(
        feats,
        np.asarray(a, np.float32),
        np.asarray(W, np.float32),
        np.asarray(Wc, np.float32),
        np.asarray(Wh, np.float32),
        np.asarray(W1, np.float32),
        np.asarray(b1, np.float32),
        np.asarray(W2, np.float32),
        np.asarray(b2, np.float32),
    )
    if "nc" not in _CACHE:
        _CACHE["nc"] = build_nc()
    res = bass_utils.run_bass_kernel_spmd(
        _CACHE["nc"], in_maps, core_ids=list(range(NCORES))
    )
    _CACHE["last_result"] = res
    out = np.concatenate([r["out"].T for r in res.results], axis=0)  # [B,7]
    return out[:, None, :].astype(np.float32)                        # [B,1,7]
